# revision 8
# baseline (speedup 1.0000x reference)
"""Multi-head attention (B=2, T=2048, H=2048, 16 heads) on 8 TRN2 NeuronCores.

Sharding: 8-way tensor parallel over heads; each core processes BOTH batches
for its 2 heads.  Core c:
  - computes qT/kT [256d, 4096t] and v [4096t, 256d] (bf16 matmuls, fp32
    PSUM) for heads {2c, 2c+1} from xT = [x[0].T | x[1].T],
  - runs softmax(q k^T / sqrt(128)) v per (batch, head) in transposed layout:
      sT = kT-chunk.T @ qT -> exp on ACT -> bf16 tiles,
      row-sums via ones-matmul, PV accumulates attn_outT [128d, 512q],
      normalization via a recip-broadcast ones-matmul + DVE multiply,
  - one 8-way AllGather of attn_outT [256, 4096] bf16 -> [2048, 4096],
  - computes o[all 4096 tokens, its 256 output columns] with its host-provided
    Wo.T column shard (per-core asymmetry lives in the *data*; the program is
    SPMD-symmetric), written as [4096, 256] fp32.
Host reassembles: out[b, :, 256c:256(c+1)] = core c rows [b*2048:(b+1)*2048].
"""

import math

import numpy as np
import ml_dtypes

import concourse.bass as bass
import concourse.tile as tile
from concourse import bacc, mybir
from concourse import bass_utils

N_CORES = 8
B = 2
T = 2048
BT = B * T        # 4096 tokens across batches
H = 2048          # hidden
HEADS = 16
HD = 128          # head dim
HPC = 2           # heads per core
DPC = HPC * HD    # projection out dims per core = 256
MS = H // N_CORES  # output-column slice per core = 256
HC = H // 128     # hidden chunks = 16
TB = 512          # token block
NTB = BT // TB    # 8
KT = T // 128     # key tiles per batch = 16
SCALE = 1.0 / math.sqrt(HD)

f32 = mybir.dt.float32
f32r = mybir.dt.float32r
bf16 = mybir.dt.bfloat16
Exp = mybir.ActivationFunctionType.Exp

_CACHE = {}
TRACE = False
LAST_RESULT = None


def _build():
    if "nc" in _CACHE:
        return _CACHE["nc"]
    nc = bacc.Bacc("TRN2", target_bir_lowering=False, debug=False,
                   num_devices=N_CORES)

    xT_d = nc.dram_tensor("xT", [H, BT], bf16, kind="ExternalInput")
    wqT_d = nc.dram_tensor("wqT", [H, DPC], bf16, kind="ExternalInput")
    wkT_d = nc.dram_tensor("wkT", [H, DPC], bf16, kind="ExternalInput")
    wvT_d = nc.dram_tensor("wvT", [H, DPC], bf16, kind="ExternalInput")
    woT_d = nc.dram_tensor("woT", [H, MS], bf16, kind="ExternalInput")
    out_d = nc.dram_tensor("out", [BT, MS], f32, kind="ExternalOutput")

    groups = [list(range(N_CORES))]

    with tile.TileContext(nc) as tc:
        with (
            tc.tile_pool(name="consts", bufs=1) as consts,
            tc.tile_pool(name="wpool", bufs=1) as wpool,
            tc.tile_pool(name="xs", bufs=20) as xs_pool,
            tc.tile_pool(name="qk", bufs=4) as qk_pool,
            tc.tile_pool(name="vp", bufs=32) as v_pool,
            tc.tile_pool(name="ex", bufs=16) as ex_pool,
            tc.tile_pool(name="sm", bufs=2) as sm_pool,
            tc.tile_pool(name="ot", bufs=4) as ot_pool,
            tc.tile_pool(name="at", bufs=24) as at_pool,
            tc.tile_pool(name="ob", bufs=3) as ob_pool,
            tc.tile_pool(name="ps_qkv", bufs=3, space="PSUM") as ps_qkv,
            tc.tile_pool(name="ps_s", bufs=2, space="PSUM") as ps_s,
            tc.tile_pool(name="ps_sums", bufs=1, space="PSUM") as ps_sums,
            tc.tile_pool(name="ps_pv", bufs=2, space="PSUM") as ps_pv,
            tc.tile_pool(name="dram", bufs=1, space="DRAM") as dram,
        ):
            # constants
            ones_col = consts.tile([128, 1], bf16)   # lhsT for row-sums
            nc.vector.memset(ones_col[:], 1.0)
            ones_row = consts.tile([1, 128], f32)    # lhsT for recip broadcast
            nc.vector.memset(ones_row[:], 1.0)
            ones_row_r = consts.tile([1, 128], f32r)
            nc.vector.tensor_copy(ones_row_r[:], ones_row[:])

            # weights
            wqT = wpool.tile([128, HC * DPC], bf16)
            wkT = wpool.tile([128, HC * DPC], bf16)
            wvT = wpool.tile([128, HC * DPC], bf16)
            woT = wpool.tile([128, HC * MS], bf16)
            for c in range(HC):
                nc.sync.dma_start(wqT[:, c * DPC:(c + 1) * DPC],
                                  wqT_d.ap()[c * 128:(c + 1) * 128, :])
                nc.sync.dma_start(wkT[:, c * DPC:(c + 1) * DPC],
                                  wkT_d.ap()[c * 128:(c + 1) * 128, :])
                nc.sync.dma_start(wvT[:, c * DPC:(c + 1) * DPC],
                                  wvT_d.ap()[c * 128:(c + 1) * 128, :])
                nc.sync.dma_start(woT[:, c * MS:(c + 1) * MS],
                                  woT_d.ap()[c * 128:(c + 1) * 128, :])

            # projection-phase outputs (live through attention)
            qkT = [qk_pool.tile([128, BT], bf16, tag="qk", name=f"qkT{i}")
                   for i in range(2 * HPC)]          # 2 q tiles then 2 k tiles
            v_sb = [v_pool.tile([128, DPC], bf16, tag="v", name=f"v{i}")
                    for i in range(BT // 128)]       # [token-tile, 256 d]

            # ---------------- QKV projections ----------------
            for tb in range(NTB):
                xt = []
                for c in range(HC):
                    xc = xs_pool.tile([128, TB], bf16, tag="xt",
                                      name=f"x{tb}_{c}")
                    nc.sync.dma_start(
                        xc[:], xT_d.ap()[c * 128:(c + 1) * 128,
                                         tb * TB:(tb + 1) * TB])
                    xt.append(xc)
                # q then k d-tiles: accumulate over hidden chunks
                for dt in range(2 * HPC):
                    w = wqT if dt < HPC else wkT
                    di = dt % HPC
                    acc = ps_qkv.tile([128, TB], f32, tag="qkv")
                    for c in range(HC):
                        nc.tensor.matmul(
                            acc[:],
                            w[:, c * DPC + di * 128: c * DPC + (di + 1) * 128],
                            xt[c][:],
                            start=(c == 0), stop=(c == HC - 1),
                        )
                    nc.scalar.copy(qkT[dt][:, tb * TB:(tb + 1) * TB], acc[:])
                # v token-tiles
                for tt in range(TB // 128):
                    acc = ps_qkv.tile([128, DPC], f32, tag="qkv")
                    for c in range(HC):
                        nc.tensor.matmul(
                            acc[:],
                            xt[c][:, tt * 128:(tt + 1) * 128],
                            wvT[:, c * DPC:(c + 1) * DPC],
                            start=(c == 0), stop=(c == HC - 1),
                        )
                    nc.vector.tensor_copy(v_sb[tb * (TB // 128) + tt][:], acc[:])

            # ---------------- attention ----------------
            cc_in = dram.tile([DPC, BT], bf16)
            cc_out = dram.tile([N_CORES * DPC, BT], bf16)

            for b in range(B):
                for h in range(HPC):
                    qT = qkT[h]
                    kT = qkT[HPC + h]
                    for qb in range(T // TB):
                        qs = qT[:, b * T + qb * TB: b * T + (qb + 1) * TB]
                        sums = ps_sums.tile([1, TB], f32, tag="sums")
                        pv = ps_pv.tile([128, TB], f32, tag="pv")
                        for kt in range(KT):
                            s_ps = ps_s.tile([128, TB], f32, tag="s")
                            nc.tensor.matmul(
                                s_ps[:],
                                kT[:, b * T + kt * 128: b * T + (kt + 1) * 128],
                                qs,
                                start=True, stop=True,
                            )
                            e = ex_pool.tile([128, TB], bf16, tag="exp",
                                             name=f"e{b}_{h}_{qb}_{kt}")
                            nc.scalar.activation(e[:], s_ps[:], Exp, scale=SCALE)
                            nc.tensor.matmul(
                                sums[:], ones_col[:], e[:],
                                start=(kt == 0), stop=(kt == KT - 1),
                            )
                            nc.tensor.matmul(
                                pv[:],
                                v_sb[b * KT + kt][:, h * 128:(h + 1) * 128],
                                e[:],
                                start=(kt == 0), stop=(kt == KT - 1),
                            )
                        sums_sb = sm_pool.tile([1, TB], f32, tag="sums_sb")
                        nc.vector.tensor_copy(sums_sb[:], sums[:])
                        recip = sm_pool.tile([1, TB], f32, tag="recip")
                        nc.vector.reciprocal(recip[:], sums_sb[:])
                        recip_r = sm_pool.tile([1, TB], f32r, tag="recip_r")
                        nc.vector.tensor_copy(recip_r[:], recip[:])
                        bc_ps = ps_s.tile([128, TB], f32, tag="s")
                        nc.tensor.matmul(
                            bc_ps[:], ones_row_r[:], recip_r[:],
                            start=True, stop=True,
                        )
                        bc_sb = sm_pool.tile([128, TB], f32, tag="bc")
                        nc.scalar.copy(bc_sb[:], bc_ps[:])
                        oT = ot_pool.tile([128, TB], bf16, tag="outT")
                        nc.vector.tensor_mul(oT[:], pv[:], bc_sb[:])
                        nc.sync.dma_start(
                            cc_in[h * 128:(h + 1) * 128,
                                  b * T + qb * TB: b * T + (qb + 1) * TB],
                            oT[:])

            # ---------------- 8-way AllGather over heads ----------------
            nc.gpsimd.collective_compute(
                "AllGather",
                mybir.AluOpType.bypass,
                replica_groups=groups,
                ins=[cc_in.opt()],
                outs=[cc_out.opt()],
            )

            # ---------------- output projection (column shard) ----------------
            for tw in range(NTB):  # 8 windows of 512 tokens
                at = []
                for c in range(HC):
                    a = at_pool.tile([128, TB], bf16, tag="at",
                                     name=f"at{tw}_{c}")
                    nc.sync.dma_start(
                        a[:], cc_out[c * 128:(c + 1) * 128,
                                     tw * TB:(tw + 1) * TB])
                    at.append(a)
                for tt in range(TB // 128):
                    o_ps = ps_s.tile([128, MS], f32, tag="s")
                    for c in range(HC):
                        nc.tensor.matmul(
                            o_ps[:],
                            at[c][:, tt * 128:(tt + 1) * 128],
                            woT[:, c * MS:(c + 1) * MS],
                            start=(c == 0), stop=(c == HC - 1),
                        )
                    o_sb = ob_pool.tile([128, MS], f32, tag="ob")
                    nc.vector.tensor_copy(o_sb[:], o_ps[:])
                    nc.sync.dma_start(
                        out_d.ap()[(tw * (TB // 128) + tt) * 128:
                                   (tw * (TB // 128) + tt + 1) * 128, :],
                        o_sb[:])

    nc.compile()
    _CACHE["nc"] = nc
    return nc


def kernel(x, Wq, Wk, Wv, Wo):
    x = np.asarray(x, dtype=np.float32)
    nc = _build()
    xT = np.ascontiguousarray(
        np.concatenate([x[0].T, x[1].T], axis=1)).astype(ml_dtypes.bfloat16)
    woT_full = np.ascontiguousarray(np.asarray(Wo).T)  # [H in(hd), H out(m)]
    in_maps = []
    for c in range(N_CORES):
        in_maps.append({
            "xT": xT,
            "wqT": np.ascontiguousarray(
                np.asarray(Wq)[c * DPC:(c + 1) * DPC, :].T
            ).astype(ml_dtypes.bfloat16),
            "wkT": np.ascontiguousarray(
                np.asarray(Wk)[c * DPC:(c + 1) * DPC, :].T
            ).astype(ml_dtypes.bfloat16),
            "wvT": np.ascontiguousarray(
                np.asarray(Wv)[c * DPC:(c + 1) * DPC, :].T
            ).astype(ml_dtypes.bfloat16),
            "woT": np.ascontiguousarray(
                woT_full[:, c * MS:(c + 1) * MS]).astype(ml_dtypes.bfloat16),
        })
    res = bass_utils.run_bass_kernel_spmd(
        nc, in_maps, core_ids=list(range(N_CORES)), trace=TRACE)
    global LAST_RESULT
    LAST_RESULT = res
    out = np.empty((B, T, H), dtype=np.float32)
    for c in range(N_CORES):
        o = res.results[c]["out"]
        for b in range(B):
            out[b, :, c * MS:(c + 1) * MS] = o[b * T:(b + 1) * T, :]
    return out


# revision 9
# speedup vs baseline: 1.0988x; 1.0988x over previous
"""Multi-head attention (B=2, T=2048, H=2048, 16 heads) on 8 TRN2 NeuronCores.

Sharding: 8-way tensor parallel over heads; each core processes BOTH batches
for its 2 heads.  Core c:
  - computes qT/kT/vT [256d, 4096t] (bf16 matmuls, fp32 PSUM accumulate) for
    heads {2c, 2c+1} from xT = [x[0].T | x[1].T]; vT is transposed to natural
    v [4096t, 256d] tiles with identity matmuls (keeps every projection
    matmul at N=512 so LDWEIGHTS hides in the PE reorder window),
  - runs softmax(q k^T / sqrt(128)) v per (batch, head) in transposed layout:
      sT = kT-chunk.T @ qT -> exp on ACT -> bf16 tiles,
      row-sums via ones-matmul, PV accumulates attn_outT [128d, 512q],
      normalization via a recip-broadcast ones-matmul + DVE multiply,
  - one 8-way AllGather of attn_outT [256, 4096] bf16 -> [2048, 4096] Shared,
  - computes oT[its 256 output columns, all 4096 tokens] with its
    host-provided Wo.T column shard as stationary (per-core asymmetry lives
    in the *data*; the program is SPMD-symmetric), written as [256, 4096]
    fp32; the host transposes back.
Host reassembles: out[b, :, 256c:256(c+1)] = core c result[:, b*T:(b+1)*T].T
"""

import math

import numpy as np
import ml_dtypes

import concourse.bass as bass
import concourse.tile as tile
from concourse import bacc, mybir
from concourse import bass_utils
from concourse.masks import make_identity

N_CORES = 8
B = 2
T = 2048
BT = B * T        # 4096 tokens across batches
H = 2048          # hidden
HEADS = 16
HD = 128          # head dim
HPC = 2           # heads per core
DPC = HPC * HD    # projection out dims per core = 256
MS = H // N_CORES  # output-column slice per core = 256
HC = H // 128     # hidden chunks = 16
TB = 512          # token block
NTB = BT // TB    # 8
KT = T // 128     # key tiles per batch = 16
SCALE = 1.0 / math.sqrt(HD)

f32 = mybir.dt.float32
f32r = mybir.dt.float32r
bf16 = mybir.dt.bfloat16
Exp = mybir.ActivationFunctionType.Exp

_CACHE = {}
TRACE = False
LAST_RESULT = None


def _build():
    if "nc" in _CACHE:
        return _CACHE["nc"]
    nc = bacc.Bacc("TRN2", target_bir_lowering=False, debug=False,
                   num_devices=N_CORES)

    xT_d = nc.dram_tensor("xT", [H, BT], bf16, kind="ExternalInput")
    wqT_d = nc.dram_tensor("wqT", [H, DPC], bf16, kind="ExternalInput")
    wkT_d = nc.dram_tensor("wkT", [H, DPC], bf16, kind="ExternalInput")
    wvT_d = nc.dram_tensor("wvT", [H, DPC], bf16, kind="ExternalInput")
    woT_d = nc.dram_tensor("woT", [H, MS], bf16, kind="ExternalInput")
    out_d = nc.dram_tensor("out", [MS, BT], f32, kind="ExternalOutput")

    groups = [list(range(N_CORES))]

    with tile.TileContext(nc) as tc:
        with (
            tc.tile_pool(name="consts", bufs=1) as consts,
            tc.tile_pool(name="wpool", bufs=1) as wpool,
            tc.tile_pool(name="xs", bufs=20) as xs_pool,
            tc.tile_pool(name="qk", bufs=6) as qk_pool,
            tc.tile_pool(name="vp", bufs=32) as v_pool,
            tc.tile_pool(name="ex", bufs=18) as ex_pool,
            tc.tile_pool(name="sm", bufs=2) as sm_pool,
            tc.tile_pool(name="ot", bufs=4) as ot_pool,
            tc.tile_pool(name="at", bufs=24) as at_pool,
            tc.tile_pool(name="ob", bufs=3) as ob_pool,
            tc.tile_pool(name="ps", bufs=1, space="PSUM") as ps,
            tc.tile_pool(name="dram", bufs=1, space="DRAM") as dram,
        ):
            # constants
            ones_col = consts.tile([128, 1], bf16)   # lhsT for row-sums
            nc.vector.memset(ones_col[:], 1.0)
            ones_row = consts.tile([1, 128], f32)    # lhsT for recip broadcast
            nc.vector.memset(ones_row[:], 1.0)
            ones_row_r = consts.tile([1, 128], f32r)
            nc.vector.tensor_copy(ones_row_r[:], ones_row[:])
            ident = consts.tile([128, 128], bf16)    # rhs for vT -> v
            make_identity(nc, ident[:])

            # weights
            wqT = wpool.tile([128, HC * DPC], bf16)
            wkT = wpool.tile([128, HC * DPC], bf16)
            wvT = wpool.tile([128, HC * DPC], bf16)
            woT = wpool.tile([128, HC * MS], bf16)
            for c in range(HC):
                nc.sync.dma_start(wqT[:, c * DPC:(c + 1) * DPC],
                                  wqT_d.ap()[c * 128:(c + 1) * 128, :])
                nc.sync.dma_start(wkT[:, c * DPC:(c + 1) * DPC],
                                  wkT_d.ap()[c * 128:(c + 1) * 128, :])
                nc.sync.dma_start(wvT[:, c * DPC:(c + 1) * DPC],
                                  wvT_d.ap()[c * 128:(c + 1) * 128, :])
                nc.sync.dma_start(woT[:, c * MS:(c + 1) * MS],
                                  woT_d.ap()[c * 128:(c + 1) * 128, :])

            # projection-phase outputs (live through attention)
            qkT = [qk_pool.tile([128, BT], bf16, tag="qk", name=f"qkT{i}")
                   for i in range(3 * HPC)]    # q0,q1,k0,k1,vT0,vT1
            v_sb = [v_pool.tile([128, DPC], bf16, tag="v", name=f"v{i}")
                    for i in range(BT // 128)]  # natural v [token-tile, 256d]

            # ---------------- QKV projections ----------------
            for tb in range(NTB):
                xt = []
                for c in range(HC):
                    xc = xs_pool.tile([128, TB], bf16, tag="xt",
                                      name=f"x{tb}_{c}")
                    nc.sync.dma_start(
                        xc[:], xT_d.ap()[c * 128:(c + 1) * 128,
                                         tb * TB:(tb + 1) * TB])
                    xt.append(xc)
                # q, k, vT d-tiles: accumulate over hidden chunks (all N=512)
                for dt in range(3 * HPC):
                    w = (wqT, wkT, wvT)[dt // HPC]
                    di = dt % HPC
                    acc = ps.tile([128, TB], f32, tag="acc", bufs=2)
                    for c in range(HC):
                        nc.tensor.matmul(
                            acc[:],
                            w[:, c * DPC + di * 128: c * DPC + (di + 1) * 128],
                            xt[c][:],
                            start=(c == 0), stop=(c == HC - 1),
                        )
                    nc.scalar.copy(qkT[2 * (dt // HPC) + di]
                                   [:, tb * TB:(tb + 1) * TB], acc[:])
                # transpose vT -> natural v for this token block
                for h in range(HPC):
                    vT = qkT[4 + h]
                    for tt in range(TB // 128):
                        gt = tb * (TB // 128) + tt   # global token tile
                        tp = ps.tile([128, 128], f32, tag="s", bufs=3,
                                     name="tp")
                        nc.tensor.matmul(
                            tp[:], vT[:, gt * 128:(gt + 1) * 128], ident[:],
                            start=True, stop=True,
                        )
                        nc.vector.tensor_copy(
                            v_sb[gt][:, h * 128:(h + 1) * 128], tp[:])

            # ---------------- attention ----------------
            cc_in = dram.tile([DPC, BT], bf16)
            cc_out = dram.tile([N_CORES * DPC, BT], bf16, addr_space="Shared")

            for b in range(B):
                for h in range(HPC):
                    qT = qkT[h]
                    kT = qkT[2 + h]
                    for qb in range(T // TB):
                        qs = qT[:, b * T + qb * TB: b * T + (qb + 1) * TB]
                        sums = ps.tile([1, TB], f32, tag="sums", bufs=1)
                        pv = ps.tile([128, TB], f32, tag="pv", bufs=2)
                        for kt in range(KT):
                            s_ps = ps.tile([128, TB], f32, tag="s", bufs=3)
                            nc.tensor.matmul(
                                s_ps[:],
                                kT[:, b * T + kt * 128: b * T + (kt + 1) * 128],
                                qs,
                                start=True, stop=True,
                            )
                            e = ex_pool.tile([128, TB], bf16, tag="exp",
                                             name=f"e{b}_{h}_{qb}_{kt}")
                            nc.scalar.activation(e[:], s_ps[:], Exp,
                                                 scale=SCALE)
                            nc.tensor.matmul(
                                sums[:], ones_col[:], e[:],
                                start=(kt == 0), stop=(kt == KT - 1),
                            )
                            nc.tensor.matmul(
                                pv[:],
                                v_sb[b * KT + kt][:, h * 128:(h + 1) * 128],
                                e[:],
                                start=(kt == 0), stop=(kt == KT - 1),
                            )
                        sums_sb = sm_pool.tile([1, TB], f32, tag="sums_sb")
                        nc.scalar.copy(sums_sb[:], sums[:])
                        recip = sm_pool.tile([1, TB], f32, tag="recip")
                        nc.vector.reciprocal(recip[:], sums_sb[:])
                        recip_r = sm_pool.tile([1, TB], f32r, tag="recip_r")
                        nc.vector.tensor_copy(recip_r[:], recip[:])
                        bc_ps = ps.tile([128, TB], f32, tag="s", bufs=3)
                        nc.tensor.matmul(
                            bc_ps[:], ones_row_r[:], recip_r[:],
                            start=True, stop=True,
                        )
                        bc_sb = sm_pool.tile([128, TB], f32, tag="bc")
                        nc.scalar.copy(bc_sb[:], bc_ps[:])
                        oT = ot_pool.tile([128, TB], bf16, tag="outT")
                        nc.vector.tensor_mul(oT[:], pv[:], bc_sb[:])
                        nc.sync.dma_start(
                            cc_in[h * 128:(h + 1) * 128,
                                  b * T + qb * TB: b * T + (qb + 1) * TB],
                            oT[:])

            # ---------------- 8-way AllGather over heads ----------------
            nc.gpsimd.collective_compute(
                "AllGather",
                mybir.AluOpType.bypass,
                replica_groups=groups,
                ins=[cc_in.opt()],
                outs=[cc_out.opt()],
            )

            # ------------- output projection (transposed, col shard) -------
            for tw in range(NTB):  # 8 windows of 512 tokens
                at = []
                for c in range(HC):
                    a = at_pool.tile([128, TB], bf16, tag="at",
                                     name=f"at{tw}_{c}")
                    nc.sync.dma_start(
                        a[:], cc_out[c * 128:(c + 1) * 128,
                                     tw * TB:(tw + 1) * TB])
                    at.append(a)
                for mb in range(MS // 128):
                    o_ps = ps.tile([128, TB], f32, tag="acc", bufs=2)
                    for c in range(HC):
                        nc.tensor.matmul(
                            o_ps[:],
                            woT[:, c * MS + mb * 128: c * MS + (mb + 1) * 128],
                            at[c][:],
                            start=(c == 0), stop=(c == HC - 1),
                        )
                    o_sb = ob_pool.tile([128, TB], f32, tag="ob")
                    nc.vector.tensor_copy(o_sb[:], o_ps[:])
                    nc.sync.dma_start(
                        out_d.ap()[mb * 128:(mb + 1) * 128,
                                   tw * TB:(tw + 1) * TB],
                        o_sb[:])

    nc.compile()
    _CACHE["nc"] = nc
    return nc


def kernel(x, Wq, Wk, Wv, Wo):
    x = np.asarray(x, dtype=np.float32)
    nc = _build()
    xT = np.ascontiguousarray(
        np.concatenate([x[0].T, x[1].T], axis=1)).astype(ml_dtypes.bfloat16)
    woT_full = np.ascontiguousarray(np.asarray(Wo).T)  # [H in(hd), H out(m)]
    in_maps = []
    for c in range(N_CORES):
        in_maps.append({
            "xT": xT,
            "wqT": np.ascontiguousarray(
                np.asarray(Wq)[c * DPC:(c + 1) * DPC, :].T
            ).astype(ml_dtypes.bfloat16),
            "wkT": np.ascontiguousarray(
                np.asarray(Wk)[c * DPC:(c + 1) * DPC, :].T
            ).astype(ml_dtypes.bfloat16),
            "wvT": np.ascontiguousarray(
                np.asarray(Wv)[c * DPC:(c + 1) * DPC, :].T
            ).astype(ml_dtypes.bfloat16),
            "woT": np.ascontiguousarray(
                woT_full[:, c * MS:(c + 1) * MS]).astype(ml_dtypes.bfloat16),
        })
    res = bass_utils.run_bass_kernel_spmd(
        nc, in_maps, core_ids=list(range(N_CORES)), trace=TRACE)
    global LAST_RESULT
    LAST_RESULT = res
    out = np.empty((B, T, H), dtype=np.float32)
    for c in range(N_CORES):
        o = res.results[c]["out"]  # [MS, BT]
        for b in range(B):
            out[b, :, c * MS:(c + 1) * MS] = o[:, b * T:(b + 1) * T].T
    return out


# revision 11
# speedup vs baseline: 1.2832x; 1.1679x over previous
"""Multi-head attention (B=2, T=2048, H=2048, 16 heads) on 8 TRN2 NeuronCores.

Sharding: 8-way tensor parallel over heads; each core processes BOTH batches
for its 2 heads.  Core c:
  - computes qT/kT/vT [256d, 4096t] (bf16 matmuls, fp32 PSUM accumulate) for
    heads {2c, 2c+1} from xT = [x[0].T | x[1].T]; vT is transposed to natural
    v [4096t, 256d] tiles with identity matmuls (keeps every projection
    matmul at N=512 so LDWEIGHTS hides in the PE reorder window),
  - runs softmax(q k^T / sqrt(128)) v per (batch, head) in transposed layout:
      sT = kT-chunk.T @ qT -> exp on ACT -> bf16 tiles,
      row-sums via ones-matmul (software-pipelined one k-tile behind the
      scores so the in-order PE queue never waits on ACT),
      normalization via a recip-broadcast ones-matmul + DVE multiply,
  - one 8-way AllGather per batch of attn_outT [256, 2048] bf16 (the batch-0
    gather overlaps batch-1 attention),
  - computes oT[its 256 output columns, all tokens] with its host-provided
    Wo.T column shard as stationary (per-core asymmetry lives in the *data*;
    the program is SPMD-symmetric), written as [256, 4096] fp32; the host
    transposes back.
DMA is spread across both HWDGE queues (sync + scalar) and all bulk loads use
full-row tiles (8KB contiguous segments).
Host reassembles: out[b, :, 256c:256(c+1)] = core c result[:, b*T:(b+1)*T].T
"""

import math

import numpy as np
import ml_dtypes

import concourse.bass as bass
import concourse.tile as tile
from concourse import bacc, mybir
from concourse import bass_utils
from concourse.masks import make_identity

N_CORES = 8
B = 2
T = 2048
BT = B * T        # 4096 tokens across batches
H = 2048          # hidden
HEADS = 16
HD = 128          # head dim
HPC = 2           # heads per core
DPC = HPC * HD    # projection out dims per core = 256
MS = H // N_CORES  # output-column slice per core = 256
HC = H // 128     # hidden chunks = 16
TB = 512          # token block
NTB = BT // TB    # 8
KT = T // 128     # key tiles per batch = 16
WSTR = 3 * DPC + MS     # 1024: per-chunk weight stride (q|k|v|o)
SCALE = 1.0 / math.sqrt(HD)

f32 = mybir.dt.float32
f32r = mybir.dt.float32r
bf16 = mybir.dt.bfloat16
Exp = mybir.ActivationFunctionType.Exp

_CACHE = {}
TRACE = False
LAST_RESULT = None


def _build():
    if "nc" in _CACHE:
        return _CACHE["nc"]
    nc = bacc.Bacc("TRN2", target_bir_lowering=False, debug=False,
                   num_devices=N_CORES)

    xT_d = nc.dram_tensor("xT", [H, BT], bf16, kind="ExternalInput")
    # all four weights interleaved per hidden chunk in SBUF layout:
    # [128, HC * (DPC+DPC+DPC+MS)] -> chunk c at columns c*WSTride
    w_all_d = nc.dram_tensor("w_all", [128, HC * WSTR], bf16,
                             kind="ExternalInput")
    out_d = nc.dram_tensor("out", [MS, BT], f32, kind="ExternalOutput")

    groups = [list(range(N_CORES))]
    dma_engs = (nc.sync, nc.scalar)  # two HWDGE queues

    with tile.TileContext(nc) as tc:
        with (
            tc.tile_pool(name="consts", bufs=1) as consts,
            tc.tile_pool(name="wpool", bufs=1) as wpool,
            tc.tile_pool(name="vp", bufs=32) as v_pool,
            tc.tile_pool(name="sm", bufs=2) as sm_pool,
            tc.tile_pool(name="ot", bufs=4) as ot_pool,
            tc.tile_pool(name="ob", bufs=3) as ob_pool,
            tc.tile_pool(name="ps", bufs=1, space="PSUM") as ps,
            tc.tile_pool(name="dram", bufs=1, space="DRAM") as dram,
        ):
            # constants
            ones_col = consts.tile([128, 1], bf16)   # lhsT for row-sums
            nc.vector.memset(ones_col[:], 1.0)
            ones_row = consts.tile([1, 128], f32)    # lhsT for recip broadcast
            nc.vector.memset(ones_row[:], 1.0)
            ones_row_r = consts.tile([1, 128], f32r)
            nc.vector.tensor_copy(ones_row_r[:], ones_row[:])
            ident = consts.tile([128, 128], bf16)    # rhs for vT -> v
            make_identity(nc, ident[:])

            # one interleaved weight tile; big contiguous DMA on gpsimd
            w_all = wpool.tile([128, HC * WSTR], bf16)
            for half in range(2):
                sl = slice(half * (HC // 2) * WSTR, (half + 1) * (HC // 2) * WSTR)
                nc.gpsimd.dma_start(w_all[:, sl], w_all_d.ap()[:, sl])

            def wq_sl(c, di):
                o = c * WSTR + di * 128
                return w_all[:, o:o + 128]

            def wk_sl(c, di):
                o = c * WSTR + DPC + di * 128
                return w_all[:, o:o + 128]

            def wv_sl(c, di):
                o = c * WSTR + 2 * DPC + di * 128
                return w_all[:, o:o + 128]

            def wo_sl(c, mb):
                o = c * WSTR + 3 * DPC + mb * 128
                return w_all[:, o:o + 128]

            v_sb = [v_pool.tile([128, DPC], bf16, tag="v", name=f"v{i}")
                    for i in range(BT // 128)]  # natural v [token-tile, 256d]

            cc_in = [dram.tile([DPC, T], bf16, name=f"cc_in{b}")
                     for b in range(B)]
            cc_out = [dram.tile([N_CORES * DPC, T], bf16,
                                addr_space="Shared", name=f"cc_out{b}")
                      for b in range(B)]

            with (
                tc.tile_pool(name="xs", bufs=24) as xs_pool,
                tc.tile_pool(name="qk", bufs=6) as qk_pool,
                tc.tile_pool(name="ex", bufs=18) as ex_pool,
            ):
                qkT = [qk_pool.tile([128, BT], bf16, tag="qk", name=f"qkT{i}")
                       for i in range(3 * HPC)]   # q0,q1,k0,k1,vT0,vT1

                def qkv_tblock(tb):
                    # stream this token block's x chunks (two HWDGE queues)
                    xt = []
                    for c in range(HC):
                        xc = xs_pool.tile([128, TB], bf16, tag="xt",
                                          name=f"x{tb}_{c}")
                        dma_engs[c % 2].dma_start(
                            xc[:], xT_d.ap()[c * 128:(c + 1) * 128,
                                             tb * TB:(tb + 1) * TB])
                        xt.append(xc)
                    # q, k, vT d-tiles for token block tb (all N=512)
                    for dt in range(3 * HPC):
                        wsl = (wq_sl, wk_sl, wv_sl)[dt // HPC]
                        di = dt % HPC
                        acc = ps.tile([128, TB], f32, tag="acc", bufs=2,
                                      name="acc")
                        for c in range(HC):
                            nc.tensor.matmul(
                                acc[:], wsl(c, di), xt[c][:],
                                start=(c == 0), stop=(c == HC - 1),
                            )
                        nc.scalar.copy(qkT[2 * (dt // HPC) + di]
                                       [:, tb * TB:(tb + 1) * TB], acc[:])
                    # transpose vT -> natural v for this token block
                    for h in range(HPC):
                        vT = qkT[4 + h]
                        for tt in range(TB // 128):
                            gt = tb * (TB // 128) + tt
                            tp = ps.tile([128, 128], f32, tag="s", bufs=3,
                                         name="tp")
                            nc.tensor.matmul(
                                tp[:], vT[:, gt * 128:(gt + 1) * 128],
                                ident[:], start=True, stop=True,
                            )
                            nc.vector.tensor_copy(
                                v_sb[gt][:, h * 128:(h + 1) * 128], tp[:])

                def attn_combo(b, h, qb):
                    qT = qkT[h]
                    kT = qkT[2 + h]
                    qs = qT[:, b * T + qb * TB: b * T + (qb + 1) * TB]
                    sums = ps.tile([1, TB], f32, tag="sums", bufs=1,
                                   name="sums")
                    pv = ps.tile([128, TB], f32, tag="pv", bufs=2, name="pv")
                    # software-pipelined: sums/pv for k-tile kt-1 are emitted
                    # after the scores matmul of k-tile kt, so the in-order PE
                    # queue never waits on the ACT exp.
                    es = []
                    for kt in range(KT + 1):
                        if kt < KT:
                            s_ps = ps.tile([128, TB], f32, tag="s", bufs=3,
                                           name="s_ps")
                            nc.tensor.matmul(
                                s_ps[:],
                                kT[:, b * T + kt * 128:
                                   b * T + (kt + 1) * 128],
                                qs, start=True, stop=True,
                            )
                            e = ex_pool.tile([128, TB], bf16, tag="exp",
                                             name=f"e{b}_{h}_{qb}_{kt}")
                            nc.scalar.activation(e[:], s_ps[:], Exp,
                                                 scale=SCALE)
                            es.append(e)
                        if kt > 0:
                            pe = es[kt - 1]
                            nc.tensor.matmul(
                                sums[:], ones_col[:], pe[:],
                                start=(kt == 1), stop=(kt == KT),
                            )
                            nc.tensor.matmul(
                                pv[:],
                                v_sb[b * KT + kt - 1]
                                [:, h * 128:(h + 1) * 128],
                                pe[:],
                                start=(kt == 1), stop=(kt == KT),
                            )
                    sums_sb = sm_pool.tile([1, TB], f32, tag="sums_sb")
                    nc.scalar.copy(sums_sb[:], sums[:])
                    recip = sm_pool.tile([1, TB], f32, tag="recip")
                    nc.vector.reciprocal(recip[:], sums_sb[:])
                    recip_r = sm_pool.tile([1, TB], f32r, tag="recip_r")
                    nc.vector.tensor_copy(recip_r[:], recip[:])
                    bc_ps = ps.tile([128, TB], f32, tag="s", bufs=3,
                                    name="bc_ps")
                    nc.tensor.matmul(bc_ps[:], ones_row_r[:], recip_r[:],
                                     start=True, stop=True)
                    bc_sb = sm_pool.tile([128, TB], f32, tag="bc")
                    nc.scalar.copy(bc_sb[:], bc_ps[:])
                    oT = ot_pool.tile([128, TB], bf16, tag="outT")
                    nc.vector.tensor_mul(oT[:], pv[:], bc_sb[:])
                    nc.sync.dma_start(
                        cc_in[b][h * 128:(h + 1) * 128,
                                 qb * TB:(qb + 1) * TB],
                        oT[:])

                # ---- emission: QKV b0 blocks, then attention b0 interleaved
                # with QKV b1 blocks, then attention b1 ----
                for tb in range(4):
                    qkv_tblock(tb)
                combos_b0 = [(0, h, qb) for h in range(HPC)
                             for qb in range(T // TB)]
                for i, tb in enumerate(range(4, 8)):
                    attn_combo(*combos_b0[2 * i])
                    qkv_tblock(tb)
                    attn_combo(*combos_b0[2 * i + 1])
                nc.gpsimd.collective_compute(
                    "AllGather", mybir.AluOpType.bypass,
                    replica_groups=groups,
                    ins=[cc_in[0].opt()], outs=[cc_out[0].opt()],
                )
                for h in range(HPC):
                    for qb in range(T // TB):
                        attn_combo(1, h, qb)
                nc.gpsimd.collective_compute(
                    "AllGather", mybir.AluOpType.bypass,
                    replica_groups=groups,
                    ins=[cc_in[1].opt()], outs=[cc_out[1].opt()],
                )

            # ------------- output projection (transposed, col shard) -------
            with tc.tile_pool(name="at", bufs=18) as at_pool:
                for b in range(B):
                    at = []
                    for c in range(HC):
                        a = at_pool.tile([128, T], bf16, tag="at",
                                         name=f"at{b}_{c}")
                        dma_engs[c % 2].dma_start(
                            a[:], cc_out[b][c * 128:(c + 1) * 128, :])
                        at.append(a)
                    for tw in range(T // TB):
                        for mb in range(MS // 128):
                            o_ps = ps.tile([128, TB], f32, tag="acc", bufs=2,
                                           name="o_ps")
                            for c in range(HC):
                                nc.tensor.matmul(
                                    o_ps[:], wo_sl(c, mb),
                                    at[c][:, tw * TB:(tw + 1) * TB],
                                    start=(c == 0), stop=(c == HC - 1),
                                )
                            o_sb = ob_pool.tile([128, TB], f32, tag="ob")
                            nc.vector.tensor_copy(o_sb[:], o_ps[:])
                            dma_engs[(tw + mb) % 2].dma_start(
                                out_d.ap()[mb * 128:(mb + 1) * 128,
                                           b * T + tw * TB:
                                           b * T + (tw + 1) * TB],
                                o_sb[:])

    nc.compile()
    _CACHE["nc"] = nc
    return nc


def kernel(x, Wq, Wk, Wv, Wo):
    x = np.asarray(x, dtype=np.float32)
    nc = _build()
    xT = np.ascontiguousarray(
        np.concatenate([x[0].T, x[1].T], axis=1)).astype(ml_dtypes.bfloat16)
    woT_full = np.asarray(Wo).T  # [H in(hd), H out(m)]
    in_maps = []
    for c in range(N_CORES):
        # SBUF-layout weight image: per hidden chunk cc, columns are
        # [wq (256) | wk (256) | wv (256) | wo (256)] for this core's shard.
        wq = np.asarray(Wq)[c * DPC:(c + 1) * DPC, :].T  # [H, DPC]
        wk = np.asarray(Wk)[c * DPC:(c + 1) * DPC, :].T
        wv = np.asarray(Wv)[c * DPC:(c + 1) * DPC, :].T
        wo = woT_full[:, c * MS:(c + 1) * MS]            # [H, MS]
        cat = np.concatenate([wq, wk, wv, wo], axis=1)   # [H, WSTR]
        w_all = np.ascontiguousarray(
            cat.reshape(HC, 128, WSTR).transpose(1, 0, 2).reshape(
                128, HC * WSTR)).astype(ml_dtypes.bfloat16)
        in_maps.append({"xT": xT, "w_all": w_all})
    res = bass_utils.run_bass_kernel_spmd(
        nc, in_maps, core_ids=list(range(N_CORES)), trace=TRACE)
    global LAST_RESULT
    LAST_RESULT = res
    out = np.empty((B, T, H), dtype=np.float32)
    for c in range(N_CORES):
        o = res.results[c]["out"]  # [MS, BT]
        for b in range(B):
            out[b, :, c * MS:(c + 1) * MS] = o[:, b * T:(b + 1) * T].T
    return out
